# revision 1
# baseline (speedup 1.0000x reference)
"""GQA attention (B=2, S=2048, D=2048, 32 Q heads / 8 KV heads, HD=64) on 8 trn2 cores.

Sharding: tensor-parallel over heads. Core c gets Q heads [4c, 4c+4), KV head c.
Each core computes a full [B*S, D] partial of the output (its 4 heads through
o_proj); the host sums the 8 partials. No collectives.

On-chip layout tricks:
  - host passes hidden^T [D, B*S] (bf16) so every projection matmul has the
    contraction dim on partitions without any on-chip transpose;
  - scores are computed transposed (scoresT[k, q] = K^T-stationary @ Q^T-moving)
    so the PV matmul consumes exp(scoresT) directly as the moving operand;
  - V is augmented with a ones column -> PV's PSUM row 64 accumulates the
    softmax denominators for free; normalization is applied to the small
    attnT [64, q] result (reciprocal + gpsimd partition_broadcast + DVE mul);
  - attnT [hd, q] is exactly the lhsT the o_proj matmul needs.
"""

import functools

import numpy as np
import ml_dtypes

import concourse.bacc as bacc
import concourse.bass as bass
import concourse.mybir as mybir
import concourse.tile as tile
from concourse.bass_utils import run_bass_kernel_spmd

B, S, D = 2, 2048, 2048
H, KVH, HD = 32, 8, 64
NCORES = 8
QH = H // NCORES            # 4 q heads per core
ST = B * S                  # 4096 flattened rows
QHD = QH * HD               # 256 (q hd dims per core)
SCALE = 1.0 / np.sqrt(HD)

BF16 = mybir.dt.bfloat16
F32 = mybir.dt.float32

DC = D // 128               # 16 contraction chunks
SC_N = ST // 512            # 8 s-chunks for projections
KB_N = S // 128             # 16 key blocks per batch
QHALF = 1024                # q columns per attention job


def _rebalance_matmul_waits(nc):
    """walrus allows only one sync-wait on a Matmult. Tile occasionally emits
    two (psum-slot release + engine ordering) on the first matmul of an
    accumulation group. The dedicated Ldweights directly preceding the matmul
    runs on the same in-order PE queue and virtually never carries a wait, so
    shifting the surplus waits onto it preserves ordering semantics."""
    for fn in nc.m.functions:
        for blk in fn.blocks:
            insts = list(blk.instructions)
            for idx, inst in enumerate(insts):
                if type(inst).__name__ != "InstMatmult":
                    continue
                si = inst.sync_info
                waits = list(si.on_wait or []) if si else []
                if len(waits) <= 1:
                    continue
                prev = insts[idx - 1] if idx else None
                assert prev is not None and type(prev).__name__ == "InstLdweights", (
                    f"matmul {inst.name} has {len(waits)} waits but no "
                    f"preceding Ldweights (got {type(prev).__name__})")
                _shift_waits(inst, si, waits, prev)


def _shift_waits(inst, si, waits, carrier):
    psi = carrier.sync_info
    pwaits = list(psi.on_wait or []) if psi else []
    assert len(pwaits) + len(waits) - 1 <= 3, (
        f"{inst.name}: too many combined waits on carrier {carrier.name}")
    moved, kept = waits[:-1], waits[-1:]
    if psi is None:
        carrier.sync_info = type(si)(on_wait=moved, on_update=[])
    else:
        psi.on_wait = pwaits + moved
    si.on_wait = kept


def _rebalance_dma_waits(nc):
    """Same single-wait limit applies to HWDGE DMACopy / gpsimd DMA-direct
    instructions. These always read an SBUF tile written by a producer
    (DVE copy / reciprocal) a few instructions earlier; the producer's
    engine tolerates 3 waits, and since the DMA already waits on the
    producer, conditions moved onto the producer still hold when the DMA
    starts."""
    for fn in nc.m.functions:
        for blk in fn.blocks:
            insts = list(blk.instructions)
            sp_seen = {}   # sem name -> max value already awaited on SP queue
            for idx, inst in enumerate(insts):
                if type(inst).__name__ not in (
                        "InstDMACopy", "InstPartitionBroadcast"):
                    continue
                si = inst.sync_info
                waits = list(si.on_wait or []) if si else []
                is_sp = str(inst.engine) == "EngineType.SP"
                if is_sp and waits:
                    # SP executes serially: waits dominated by an earlier SP
                    # instruction's wait on the same sem are redundant
                    live = [w for w in waits
                            if sp_seen.get(w.ant_name, -1) < w.wait_value]
                    if len(live) < len(waits):
                        si.on_wait = live
                        waits = live
                if is_sp:
                    for w in waits:
                        if sp_seen.get(w.ant_name, -1) < w.wait_value:
                            sp_seen[w.ant_name] = w.wait_value
                if len(waits) <= 1:
                    continue
                src = inst.ins[0].memref if inst.ins else None
                prod = None
                for j in range(idx - 1, max(-1, idx - 400), -1):
                    p = insts[j]
                    pouts = getattr(p, "outs", None)
                    if pouts and pouts[0].memref == src and \
                            type(p).__name__ not in ("InstDMACopy",):
                        prod = p
                        break
                if prod is None:
                    # DRAM load: no producer. SP executes serially, so the
                    # nearest preceding wait-free SP DMA can absorb the
                    # engine-WAR wait; the queue wait stays on this DMA.
                    carrier = None
                    for j in range(idx - 1, max(-1, idx - 400), -1):
                        p = insts[j]
                        if type(p).__name__ == "InstDMACopy" and \
                                str(p.engine) == "EngineType.SP":
                            pw = list(p.sync_info.on_wait or []) \
                                if p.sync_info else []
                            if not pw:
                                carrier = p
                                break
                    if carrier is None:
                        # The engine-WAR wait (kept) implies the slot's
                        # previous DMA write completed (its readers waited on
                        # it), so the same-queue WAW wait is redundant.
                        keep = [w for w in waits if "DMAHW" not in w.ant_name]
                        assert len(keep) == 1, (
                            f"{inst.name}: unexpected pair "
                            f"{[(w.ant_name, w.wait_value) for w in waits]}")
                        si.on_wait = keep
                        continue
                    waits.sort(key=lambda w: 1 if "DMAHW" in w.ant_name else 0)
                    _shift_waits(inst, si, waits, carrier)
                    continue
                # keep the producer-engine wait on the DMA, move the rest
                eng = str(prod.engine)
                key = {"EngineType.DVE": "DVE", "EngineType.ACT": "Activation",
                       "EngineType.Pool": "Pool", "EngineType.PE": "PE",
                       "EngineType.SP": "SP"}.get(eng, "zz")
                waits.sort(key=lambda w: 0 if w.ant_name.startswith(key) else 1)
                waits = waits[::-1]  # producer wait last -> kept
                psi = prod.sync_info
                pn = len(list(psi.on_wait or [])) if psi else 0
                if pn + len(waits) - 1 <= 3:
                    _shift_waits(inst, si, waits, prod)
                else:
                    # producer full: queue wait is FIFO-covered (slot reuse
                    # distance is a multiple of the 8 round-robin queues)
                    keep = [w for w in waits if "DMAHW" not in w.ant_name]
                    assert len(keep) == 1, (
                        f"{inst.name}: unexpected {[(w.ant_name, w.wait_value) for w in waits]}")
                    si.on_wait = keep


def build_program(trace_friendly: bool = False):
    nc = bacc.Bacc("TRN2", target_bir_lowering=False)
    ht = nc.dram_tensor("ht", [D, ST], BF16, kind="ExternalInput")
    wq = nc.dram_tensor("wq", [D, QHD], BF16, kind="ExternalInput")
    wk = nc.dram_tensor("wk", [D, HD], BF16, kind="ExternalInput")
    wv = nc.dram_tensor("wv", [D, HD], BF16, kind="ExternalInput")
    wo = nc.dram_tensor("wo", [QHD, D], BF16, kind="ExternalInput")
    out = nc.dram_tensor("out", [ST, D], F32, kind="ExternalOutput")

    with tile.TileContext(nc) as tc:
        with (
            tc.tile_pool(name="singles", bufs=1) as singles,
            tc.tile_pool(name="hstream", bufs=2) as hstream,
            tc.tile_pool(name="expp", bufs=3) as expp,
            tc.tile_pool(name="attn", bufs=2) as attnp,
            tc.tile_pool(name="norm", bufs=2) as normp,
            tc.tile_pool(name="ostage", bufs=8) as ostage,
            tc.tile_pool(name="ps_sc", bufs=2, space="PSUM") as ps_sc,
            tc.tile_pool(name="ps_out", bufs=1, space="PSUM") as ps_out,
            tc.tile_pool(name="ps_op", bufs=2, space="PSUM") as ps_op,
        ):
            # ---- resident weights ----
            wq_sb = singles.tile([128, DC, QHD], BF16)
            wk_sb = singles.tile([128, DC, HD], BF16)
            wv_sb = singles.tile([128, DC, HD], BF16)
            wo_sb = singles.tile([128, 2, D], BF16)
            for dc in range(DC):
                nc.sync.dma_start(wq_sb[:, dc], wq[dc * 128:(dc + 1) * 128, :])
                nc.sync.dma_start(wk_sb[:, dc], wk[dc * 128:(dc + 1) * 128, :])
                nc.sync.dma_start(wv_sb[:, dc], wv[dc * 128:(dc + 1) * 128, :])
            for hh in range(2):
                nc.sync.dma_start(wo_sb[:, hh], wo[hh * 128:(hh + 1) * 128, :])
            # pad SP DMA count to a multiple of 8 so the h-load stream keeps
            # a stable queue phase (same-queue slot reuse -> waits elided)
            pad_sb = singles.tile([1, 6, 16], BF16)
            for i in range(6):
                nc.sync.dma_start(pad_sb[:, i], wo[0:1, 0:16])

            # ---- resident activations (per batch) ----
            qt_sb = [[singles.tile([64, S], BF16, tag=f"qt{h}_{b}",
                                   name=f"qt{h}_{b}")
                      for b in range(B)] for h in range(QH)]
            kt_sb = [singles.tile([64, S], BF16, tag=f"kt{b}", name=f"kt{b}")
                     for b in range(B)]
            vaug_sb = [singles.tile([128, KB_N, HD + 1], BF16, tag=f"vaug{b}",
                                    name=f"vaug{b}")
                       for b in range(B)]
            for b in range(B):
                nc.vector.memset(vaug_sb[b][:, :, HD:HD + 1], 1.0)

            # ================= phase 1: QKV projections =================
            # b-major s-chunks so batch-0 attention can start early
            for sc in range(SC_N):
                b = sc // (SC_N // B)
                scol = (sc % (SC_N // B)) * 512      # column offset within batch
                h_sb = hstream.tile([128, DC, 512], BF16)
                for dc in range(DC):
                    nc.sync.dma_start(
                        h_sb[:, dc],
                        ht[dc * 128:(dc + 1) * 128, sc * 512:(sc + 1) * 512])

                # Q^T: two 128-row chunks of hd
                for m in range(2):
                    pq = ps_op.tile([128, 512], F32, tag="po")
                    for dc in range(DC):
                        nc.tensor.matmul(
                            pq,
                            wq_sb[:, dc, m * 128:(m + 1) * 128],
                            h_sb[:, dc],
                            start=(dc == 0), stop=(dc == DC - 1))
                    for hs in range(2):
                        h = 2 * m + hs
                        nc.vector.tensor_copy(
                            qt_sb[h][b][:, scol:scol + 512],
                            pq[hs * 64:(hs + 1) * 64, :])

                # K^T
                pk = ps_op.tile([64, 512], F32, tag="po")
                for dc in range(DC):
                    nc.tensor.matmul(pk, wk_sb[:, dc], h_sb[:, dc],
                                     start=(dc == 0), stop=(dc == DC - 1))
                nc.vector.tensor_copy(kt_sb[b][:, scol:scol + 512], pk)

                # V (natural layout, h as stationary)
                for sb in range(4):
                    pv = ps_op.tile([128, HD], F32, tag="po")
                    for dc in range(DC):
                        nc.tensor.matmul(
                            pv, h_sb[:, dc, sb * 128:(sb + 1) * 128],
                            wv_sb[:, dc],
                            start=(dc == 0), stop=(dc == DC - 1))
                    kb = (scol // 512) * 4 + sb
                    nc.vector.tensor_copy(vaug_sb[b][:, kb, 0:HD], pv)

            # ============ phase 2: attention + o_proj ============
            for b in range(B):
                for qh in range(S // QHALF):
                    q0 = qh * QHALF                   # within-batch col offset
                    attn_sb = attnp.tile([128, 2, QHALF], BF16)
                    for h in range(QH):
                        outp = ps_out.tile([HD + 1, QHALF], F32)
                        for kb in range(KB_N):
                            scp = ps_sc.tile([128, QHALF], F32)
                            for qq in range(2):
                                nc.tensor.matmul(
                                    scp[:, qq * 512:(qq + 1) * 512],
                                    kt_sb[b][:, kb * 128:(kb + 1) * 128],
                                    qt_sb[h][b][:, q0 + qq * 512:q0 + (qq + 1) * 512],
                                    start=True, stop=True)
                            expT = expp.tile([128, QHALF], BF16)
                            nc.scalar.activation(
                                expT, scp, mybir.ActivationFunctionType.Exp,
                                scale=SCALE)
                            for qq in range(2):
                                nc.tensor.matmul(
                                    outp[:, qq * 512:(qq + 1) * 512],
                                    vaug_sb[b][:, kb],
                                    expT[:, qq * 512:(qq + 1) * 512],
                                    start=(kb == 0), stop=(kb == KB_N - 1))
                        # normalize -> attnT slice (head h occupies rows
                        # (h%2)*64..+64 of sub-tensor h//2)
                        recip = normp.tile([1, QHALF], F32, tag="recip")
                        nc.vector.reciprocal(recip, outp[HD:HD + 1, :])
                        bcast = normp.tile([64, QHALF], F32, tag="bcast")
                        nc.gpsimd.partition_broadcast(bcast, recip)
                        nc.vector.tensor_mul(
                            attn_sb[(h % 2) * 64:(h % 2) * 64 + 64, h // 2, :],
                            outp[0:HD, :], bcast)

                    # o_proj for this (b, qh) group
                    for qc in range(QHALF // 128):
                        for nb in range(D // 512):
                            po = ps_op.tile([128, 512], F32, tag="po")
                            for hh in range(2):
                                nc.tensor.matmul(
                                    po,
                                    attn_sb[:, hh, qc * 128:(qc + 1) * 128],
                                    wo_sb[:, hh, nb * 512:(nb + 1) * 512],
                                    start=(hh == 0), stop=(hh == 1))
                            osb = ostage.tile([128, 512], F32)
                            nc.vector.tensor_copy(osb, po)
                            row = b * S + q0 + qc * 128
                            nc.sync.dma_start(
                                out[row:row + 128, nb * 512:(nb + 1) * 512], osb)
    nc.compile()
    return nc


@functools.lru_cache(maxsize=1)
def _get_program():
    return build_program()


def kernel(hidden_states, Wq, Wk, Wv, Wo):
    hidden_states = np.asarray(hidden_states)
    Wq, Wk, Wv, Wo = (np.asarray(x) for x in (Wq, Wk, Wv, Wo))
    bf = ml_dtypes.bfloat16

    htT = np.ascontiguousarray(
        hidden_states.reshape(ST, D).T.astype(bf))          # [D, B*S]
    in_maps = []
    for c in range(NCORES):
        in_maps.append({
            "ht": htT,
            "wq": np.ascontiguousarray(Wq[:, c * QHD:(c + 1) * QHD].astype(bf)),
            "wk": np.ascontiguousarray(Wk[:, c * HD:(c + 1) * HD].astype(bf)),
            "wv": np.ascontiguousarray(Wv[:, c * HD:(c + 1) * HD].astype(bf)),
            "wo": np.ascontiguousarray(Wo[c * QHD:(c + 1) * QHD, :].astype(bf)),
        })

    nc = _get_program()
    res = run_bass_kernel_spmd(nc, in_maps, core_ids=list(range(NCORES)))
    total = res.results[0]["out"].astype(np.float64)
    for c in range(1, NCORES):
        total += res.results[c]["out"].astype(np.float64)
    return total.reshape(B, S, D).astype(np.float32)



# revision 3
# speedup vs baseline: 1.2440x; 1.2440x over previous
"""GQA attention (B=2, S=2048, D=2048, 32 Q heads / 8 KV heads, HD=64) on 8 trn2 cores.

Sharding: tensor-parallel over heads. Core c gets Q heads [4c, 4c+4), KV head c.
Each core computes a full [B*S, D] partial of the output (its 4 heads through
o_proj); the host sums the 8 partials. No collectives.

v2 design (vs the v1 baseline):
  - K and V projections merged into one matmul stream (stationary [wk|wv]
    [128,128]) -> K^T rows 0-63, V^T rows 64-127 of each PSUM tile. V^T is
    turned into natural V via PE transposes (the v1 h-stationary V projection
    was Ldweights-bound).
  - Scores matmuls are row-tiled pairs: contraction is HD=64, so two key
    blocks' K^T stationaries sit on partition halves (kt2[0:64]=even kb,
    kt2[64:128]=odd kb) and the two matmuls run CONCURRENTLY in the PE array
    (tile_position row groups, auto-derived from base partitions). Q is
    duplicated onto both partition halves via SBUF->SBUF DMA.
  - Normalization is job-level and decoupled: PV accumulators (PSUM) are
    drained per head to SBUF by one DVE copy, then recip -> gpsimd
    partition_broadcast -> DVE muls happen off the critical PE path
    (outp bufs=2 so the next head's PV never waits).
  - o_proj of job J is interleaved into job J+1's kb loop (and into the
    batch-1 projection phase) so the PE never idles while ACT does exp.
  - Output is written bf16 (host accumulates partials in f64).
"""

import functools

import numpy as np
import ml_dtypes

import concourse.bacc as bacc
import concourse.bass as bass
import concourse.mybir as mybir
import concourse.tile as tile
from concourse import masks
from concourse.bass_utils import run_bass_kernel_spmd

B, S, D = 2, 2048, 2048
H, KVH, HD = 32, 8, 64
NCORES = 8
QH = H // NCORES            # 4 q heads per core
ST = B * S                  # 4096 flattened rows
QHD = QH * HD               # 256 (q hd dims per core)
SCALE = 1.0 / np.sqrt(HD)

BF16 = mybir.dt.bfloat16
F32 = mybir.dt.float32

DC = D // 128               # 16 contraction chunks
SC_N = S // 512             # 4 s-chunks per batch for projections
KB_N = S // 128             # 16 key blocks per batch
KP_N = KB_N // 2            # 8 key-block pairs
QJ_N = S // 512             # 4 q-jobs of 512 per batch


def _rebalance_matmul_waits(nc):
    """walrus allows only one sync-wait on a Matmult. Tile occasionally emits
    two (psum-slot release + engine ordering) on the first matmul of an
    accumulation group. The dedicated Ldweights directly preceding the matmul
    runs on the same in-order PE queue and virtually never carries a wait, so
    shifting the surplus waits onto it preserves ordering semantics."""
    for fn in nc.m.functions:
        for blk in fn.blocks:
            insts = list(blk.instructions)
            for idx, inst in enumerate(insts):
                if type(inst).__name__ != "InstMatmult":
                    continue
                si = inst.sync_info
                waits = list(si.on_wait or []) if si else []
                if len(waits) <= 1:
                    continue
                prev = insts[idx - 1] if idx else None
                assert prev is not None and type(prev).__name__ == "InstLdweights", (
                    f"matmul {inst.name} has {len(waits)} waits but no "
                    f"preceding Ldweights (got {type(prev).__name__})")
                _shift_waits(inst, si, waits, prev)


def _shift_waits(inst, si, waits, carrier):
    psi = carrier.sync_info
    pwaits = list(psi.on_wait or []) if psi else []
    assert len(pwaits) + len(waits) - 1 <= 3, (
        f"{inst.name}: too many combined waits on carrier {carrier.name}")
    moved, kept = waits[:-1], waits[-1:]
    if psi is None:
        carrier.sync_info = type(si)(on_wait=moved, on_update=[])
    else:
        psi.on_wait = pwaits + moved
    si.on_wait = kept


def _rebalance_dma_waits(nc):
    """Same single-wait limit applies to HWDGE DMACopy / gpsimd DMA-direct
    instructions. These always read an SBUF tile written by a producer
    (DVE copy / reciprocal) a few instructions earlier; the producer's
    engine tolerates 3 waits, and since the DMA already waits on the
    producer, conditions moved onto the producer still hold when the DMA
    starts."""
    for fn in nc.m.functions:
        for blk in fn.blocks:
            insts = list(blk.instructions)
            sp_seen = {}   # sem name -> max value already awaited on SP queue
            for idx, inst in enumerate(insts):
                if type(inst).__name__ not in (
                        "InstDMACopy", "InstPartitionBroadcast"):
                    continue
                si = inst.sync_info
                waits = list(si.on_wait or []) if si else []
                is_sp = str(inst.engine) == "EngineType.SP"
                if is_sp and waits:
                    # SP executes serially: waits dominated by an earlier SP
                    # instruction's wait on the same sem are redundant
                    live = [w for w in waits
                            if sp_seen.get(w.ant_name, -1) < w.wait_value]
                    if len(live) < len(waits):
                        si.on_wait = live
                        waits = live
                if is_sp:
                    for w in waits:
                        if sp_seen.get(w.ant_name, -1) < w.wait_value:
                            sp_seen[w.ant_name] = w.wait_value
                if len(waits) <= 1:
                    continue
                src = inst.ins[0].memref if inst.ins else None
                prod = None
                for j in range(idx - 1, max(-1, idx - 400), -1):
                    p = insts[j]
                    pouts = getattr(p, "outs", None)
                    if pouts and pouts[0].memref == src and \
                            type(p).__name__ not in ("InstDMACopy",):
                        prod = p
                        break
                if prod is None:
                    # DRAM load: no producer. SP executes serially, so the
                    # nearest preceding wait-free SP DMA can absorb the
                    # engine-WAR wait; the queue wait stays on this DMA.
                    carrier = None
                    for j in range(idx - 1, max(-1, idx - 400), -1):
                        p = insts[j]
                        if type(p).__name__ == "InstDMACopy" and \
                                str(p.engine) == "EngineType.SP":
                            pw = list(p.sync_info.on_wait or []) \
                                if p.sync_info else []
                            if not pw:
                                carrier = p
                                break
                    if carrier is None:
                        # The engine-WAR wait (kept) implies the slot's
                        # previous DMA write completed (its readers waited on
                        # it), so the same-queue WAW wait is redundant.
                        keep = [w for w in waits if "DMAHW" not in w.ant_name]
                        assert len(keep) == 1, (
                            f"{inst.name}: unexpected pair "
                            f"{[(w.ant_name, w.wait_value) for w in waits]}")
                        si.on_wait = keep
                        continue
                    waits.sort(key=lambda w: 1 if "DMAHW" in w.ant_name else 0)
                    _shift_waits(inst, si, waits, carrier)
                    continue
                # keep the producer-engine wait on the DMA, move the rest
                eng = str(prod.engine)
                key = {"EngineType.DVE": "DVE", "EngineType.ACT": "Activation",
                       "EngineType.Pool": "Pool", "EngineType.PE": "PE",
                       "EngineType.SP": "SP"}.get(eng, "zz")
                waits.sort(key=lambda w: 0 if w.ant_name.startswith(key) else 1)
                waits = waits[::-1]  # producer wait last -> kept
                psi = prod.sync_info
                pn = len(list(psi.on_wait or [])) if psi else 0
                if pn + len(waits) - 1 <= 3:
                    _shift_waits(inst, si, waits, prod)
                else:
                    # producer full: queue wait is FIFO-covered (slot reuse
                    # distance is a multiple of the 8 round-robin queues)
                    keep = [w for w in waits if "DMAHW" not in w.ant_name]
                    assert len(keep) == 1, (
                        f"{inst.name}: unexpected {[(w.ant_name, w.wait_value) for w in waits]}")
                    si.on_wait = keep


def build_program(trace_friendly: bool = False):
    nc = bacc.Bacc("TRN2", target_bir_lowering=False)
    ht = nc.dram_tensor("ht", [D, ST], BF16, kind="ExternalInput")
    wq = nc.dram_tensor("wq", [D, QHD], BF16, kind="ExternalInput")
    wkv = nc.dram_tensor("wkv", [D, 2 * HD], BF16, kind="ExternalInput")
    wo = nc.dram_tensor("wo", [QHD, D], BF16, kind="ExternalInput")
    out = nc.dram_tensor("out", [ST, D], BF16, kind="ExternalOutput")

    with tile.TileContext(nc) as tc:
        with (
            tc.tile_pool(name="singles", bufs=1) as singles,
            tc.tile_pool(name="hstream", bufs=2) as hstream,
            tc.tile_pool(name="expp", bufs=3) as expp,
            tc.tile_pool(name="araw", bufs=2) as arawp,
            tc.tile_pool(name="attn", bufs=2) as attnp,
            tc.tile_pool(name="norm", bufs=2) as normp,
            tc.tile_pool(name="ostage", bufs=4) as ostage,
            tc.tile_pool(name="ps_sc", bufs=2, space="PSUM") as ps_sc,
            tc.tile_pool(name="ps_out", bufs=2, space="PSUM") as ps_out,
            tc.tile_pool(name="ps_op", bufs=2, space="PSUM") as ps_op,
        ):
            # ---- resident weights ----
            wq_sb = singles.tile([128, DC, QHD], BF16)
            wkv_sb = singles.tile([128, DC, 2 * HD], BF16)
            wo_sb = singles.tile([128, 2, D], BF16)
            for dc in range(DC):
                nc.sync.dma_start(wq_sb[:, dc], wq[dc * 128:(dc + 1) * 128, :])
                nc.sync.dma_start(wkv_sb[:, dc], wkv[dc * 128:(dc + 1) * 128, :])
            for hh in range(2):
                nc.sync.dma_start(wo_sb[:, hh], wo[hh * 128:(hh + 1) * 128, :])

            # identity for the PE V-transposes
            id_sb = singles.tile([64, HD], BF16)
            masks.make_identity(nc, id_sb[:, :])

            # ---- resident activations (per batch) ----
            # qtdup[h][b]: [128, S], Q^T duplicated on both partition halves
            qtdup = [[singles.tile([128, S], BF16, tag=f"qt{h}_{b}",
                                   name=f"qt{h}_{b}")
                      for b in range(B)] for h in range(QH)]
            # kt2[b]: [128, KP_N, 128]; rows 0:64 = even kb K^T, 64:128 = odd
            kt2 = [singles.tile([128, KP_N, 128], BF16, tag=f"kt{b}",
                                name=f"kt{b}") for b in range(B)]
            vaug = [singles.tile([128, KB_N, HD + 1], BF16, tag=f"vaug{b}",
                                 name=f"vaug{b}") for b in range(B)]
            # V^T staging: rows 64:128 written by DVE (in-partition from
            # PSUM), rows 0:64 filled by SBUF->SBUF DMA; transposed at the
            # end of the projection phase.
            vt_sb = [singles.tile([128, S], BF16, tag=f"vt{b}", name=f"vt{b}")
                     for b in range(B)]
            for b in range(B):
                nc.vector.memset(vaug[b][:, :, HD:HD + 1], 1.0)

            # pending o_proj work from the previous attention job: a list of
            # closures, each one (2 matmuls + a DVE cast [+ DMA]).
            pending = []

            def run_pending(n):
                for _ in range(min(n, len(pending))):
                    pending.pop(0)()

            def proj_phase(b):
                for sc in range(SC_N):
                    scol = sc * 512
                    h_sb = hstream.tile([128, DC, 512], BF16)
                    for dc in range(DC):
                        nc.sync.dma_start(
                            h_sb[:, dc],
                            ht[dc * 128:(dc + 1) * 128,
                               b * S + scol:b * S + scol + 512])

                    # --- merged K|V projection ---
                    pkv = ps_op.tile([128, 512], F32, tag="po")
                    for dc in range(DC):
                        nc.tensor.matmul(pkv, wkv_sb[:, dc], h_sb[:, dc],
                                         start=(dc == 0), stop=(dc == DC - 1))
                    # K^T rows 0:64 -> kt2: kb 4sc+j; even j -> low half
                    # (in-partition), odd j -> high half (cross-partition,
                    # the DVE read-low/write-high direction the v1 kernel
                    # already exercised).
                    for j in range(4):
                        kp, half = (4 * sc + j) // 2, (4 * sc + j) % 2
                        nc.vector.tensor_copy(
                            kt2[b][64 * half:64 * half + 64, kp, :],
                            pkv[0:64, j * 128:(j + 1) * 128])
                    # V^T rows 64:128 -> staging high half, then DMA down
                    nc.vector.tensor_copy(
                        vt_sb[b][64:128, scol:scol + 512], pkv[64:128, :])
                    nc.sync.dma_start(
                        vt_sb[b][0:64, scol:scol + 512],
                        vt_sb[b][64:128, scol:scol + 512])

                    # --- Q projection (2 head pairs) ---
                    for m in range(2):
                        pq = ps_op.tile([128, 512], F32, tag="po")
                        for dc in range(DC):
                            nc.tensor.matmul(
                                pq, wq_sb[:, dc, m * 128:(m + 1) * 128],
                                h_sb[:, dc],
                                start=(dc == 0), stop=(dc == DC - 1))
                        h0, h1 = 2 * m, 2 * m + 1
                        nc.vector.tensor_copy(
                            qtdup[h0][b][0:64, scol:scol + 512], pq[0:64, :])
                        nc.vector.tensor_copy(
                            qtdup[h1][b][64:128, scol:scol + 512],
                            pq[64:128, :])
                        # duplicate onto the other partition half (DMA)
                        nc.sync.dma_start(
                            qtdup[h0][b][64:128, scol:scol + 512],
                            qtdup[h0][b][0:64, scol:scol + 512])
                        nc.sync.dma_start(
                            qtdup[h1][b][0:64, scol:scol + 512],
                            qtdup[h1][b][64:128, scol:scol + 512])
                        run_pending(1)

                # --- V transposes: vt_sb low half -> natural V in vaug ---
                for g in range(4):           # groups of 4 key blocks
                    tr = ps_op.tile([128, 4, HD], BF16, tag="po")
                    for j in range(4):
                        kb = 4 * g + j
                        nc.tensor.transpose(
                            tr[:, j, :],
                            vt_sb[b][0:64, kb * 128:(kb + 1) * 128],
                            id_sb[:, :])
                    nc.vector.tensor_copy(
                        vaug[b][:, 4 * g:4 * g + 4, 0:HD], tr[:, :, :])
                    run_pending(1)

            def attn_job(b, qj):
                q0 = qj * 512
                araw = arawp.tile([65, QH, 512], BF16)
                attn_sb = attnp.tile([128, 2, 512], BF16)
                for h in range(QH):
                    outp = ps_out.tile([HD + 1, 512], F32)
                    for kp in range(KP_N):
                        scp = ps_sc.tile([128, 1024], F32)
                        nc.tensor.matmul(
                            scp[:, 0:512], kt2[b][0:64, kp, :],
                            qtdup[h][b][0:64, q0:q0 + 512],
                            start=True, stop=True)
                        nc.tensor.matmul(
                            scp[:, 512:1024], kt2[b][64:128, kp, :],
                            qtdup[h][b][64:128, q0:q0 + 512],
                            start=True, stop=True)
                        expT = expp.tile([128, 1024], BF16)
                        nc.scalar.activation(
                            expT[:, :], scp[:, :],
                            mybir.ActivationFunctionType.Exp, scale=SCALE)
                        nc.tensor.matmul(
                            outp, vaug[b][:, 2 * kp, :], expT[:, 0:512],
                            start=(kp == 0), stop=False)
                        nc.tensor.matmul(
                            outp, vaug[b][:, 2 * kp + 1, :], expT[:, 512:1024],
                            start=False, stop=(kp == KP_N - 1))
                        if kp >= 2:
                            run_pending(1)
                    # drain this head's accumulator to SBUF (frees PSUM fast)
                    nc.vector.tensor_copy(araw[:, h, :], outp)

                # job-level normalization (off the PE critical path)
                recip = normp.tile([1, QH * 512], F32, tag="recip")
                nc.vector.reciprocal(recip, araw[64:65, :, :])
                bcast = normp.tile([64, QH * 512], F32, tag="bcast")
                nc.gpsimd.partition_broadcast(bcast, recip)
                for h in range(QH):
                    nc.vector.tensor_mul(
                        attn_sb[(h % 2) * 64:(h % 2) * 64 + 64, h // 2, :],
                        araw[0:64, h, :], bcast[:, h * 512:(h + 1) * 512])

                # queue this job's o_proj as pending closures
                for qc in range(4):
                    ost = ostage.tile([128, 2048], BF16)
                    row = b * S + q0 + qc * 128
                    for nb in range(4):
                        def grp(qc=qc, nb=nb, ost=ost, row=row,
                                attn_sb=attn_sb):
                            po = ps_op.tile([128, 512], F32, tag="po")
                            for hh in range(2):
                                nc.tensor.matmul(
                                    po,
                                    attn_sb[:, hh, qc * 128:(qc + 1) * 128],
                                    wo_sb[:, hh, nb * 512:(nb + 1) * 512],
                                    start=(hh == 0), stop=(hh == 1))
                            nc.vector.tensor_copy(
                                ost[:, nb * 512:(nb + 1) * 512], po)
                            if nb == 3:
                                for dd in range(2):
                                    nc.sync.dma_start(
                                        out[row:row + 128,
                                            dd * 1024:(dd + 1) * 1024],
                                        ost[:, dd * 1024:(dd + 1) * 1024])
                        pending.append(grp)

            # ================= schedule =================
            proj_phase(0)
            for qj in range(QJ_N):
                attn_job(0, qj)
            proj_phase(1)
            for qj in range(QJ_N):
                attn_job(1, qj)
            run_pending(len(pending))
    nc.compile()
    _rebalance_matmul_waits(nc)
    _rebalance_dma_waits(nc)
    return nc


@functools.lru_cache(maxsize=1)
def _get_program():
    return build_program()


def _in_maps(hidden_states, Wq, Wk, Wv, Wo):
    bf = ml_dtypes.bfloat16
    htT = np.ascontiguousarray(
        hidden_states.reshape(ST, D).T.astype(bf))          # [D, B*S]
    in_maps = []
    for c in range(NCORES):
        wkv = np.concatenate(
            [Wk[:, c * HD:(c + 1) * HD], Wv[:, c * HD:(c + 1) * HD]], axis=1)
        in_maps.append({
            "ht": htT,
            "wq": np.ascontiguousarray(Wq[:, c * QHD:(c + 1) * QHD].astype(bf)),
            "wkv": np.ascontiguousarray(wkv.astype(bf)),
            "wo": np.ascontiguousarray(Wo[c * QHD:(c + 1) * QHD, :].astype(bf)),
        })
    return in_maps


def kernel(hidden_states, Wq, Wk, Wv, Wo):
    hidden_states = np.asarray(hidden_states)
    Wq, Wk, Wv, Wo = (np.asarray(x) for x in (Wq, Wk, Wv, Wo))
    in_maps = _in_maps(hidden_states, Wq, Wk, Wv, Wo)
    nc = _get_program()
    res = run_bass_kernel_spmd(nc, in_maps, core_ids=list(range(NCORES)))
    total = res.results[0]["out"].astype(np.float64)
    for c in range(1, NCORES):
        total += res.results[c]["out"].astype(np.float64)
    return total.reshape(B, S, D).astype(np.float32)


# revision 7
# speedup vs baseline: 1.4556x; 1.1701x over previous
"""GQA attention (B=2, S=2048, D=2048, 32 Q heads / 8 KV heads, HD=64) on 8 trn2 cores.

Sharding: tensor-parallel over heads. Core c gets Q heads [4c, 4c+4), KV head c.
Each core computes a full [B*S, D] partial of the output (its 4 heads through
o_proj); the host sums the 8 partials. No collectives.

v2 design (vs the v1 baseline):
  - K and V projections merged into one matmul stream (stationary [wk|wv]
    [128,128]) -> K^T rows 0-63, V^T rows 64-127 of each PSUM tile. V^T is
    turned into natural V via PE transposes (the v1 h-stationary V projection
    was Ldweights-bound).
  - Scores matmuls are row-tiled pairs: contraction is HD=64, so two key
    blocks' K^T stationaries sit on partition halves (kt2[0:64]=even kb,
    kt2[64:128]=odd kb) and the two matmuls run CONCURRENTLY in the PE array
    (tile_position row groups, auto-derived from base partitions). Q is
    duplicated onto both partition halves via SBUF->SBUF DMA.
  - Normalization is job-level and decoupled: PV accumulators (PSUM) are
    drained per head to SBUF by one DVE copy, then recip -> gpsimd
    partition_broadcast -> DVE muls happen off the critical PE path
    (outp bufs=2 so the next head's PV never waits).
  - o_proj of job J is interleaved into job J+1's kb loop (and into the
    batch-1 projection phase) so the PE never idles while ACT does exp.
  - Output is written bf16 (host accumulates partials in f64).
"""

import functools

import numpy as np
import ml_dtypes

import concourse.bacc as bacc
import concourse.bass as bass
import concourse.mybir as mybir
import concourse.tile as tile
from concourse import masks
from concourse.bass_utils import run_bass_kernel_spmd

B, S, D = 2, 2048, 2048
H, KVH, HD = 32, 8, 64
NCORES = 8
QH = H // NCORES            # 4 q heads per core
ST = B * S                  # 4096 flattened rows
QHD = QH * HD               # 256 (q hd dims per core)
SCALE = 1.0 / np.sqrt(HD)

BF16 = mybir.dt.bfloat16
F32 = mybir.dt.float32

DC = D // 128               # 16 contraction chunks
SC_N = S // 512             # 4 s-chunks per batch for projections
KB_N = S // 128             # 16 key blocks per batch
KP_N = KB_N // 2            # 8 key-block pairs
QJ_N = S // 512             # 4 q-jobs of 512 per batch


def _rebalance_matmul_waits(nc):
    """walrus allows only one sync-wait on a Matmult. Tile occasionally emits
    two (psum-slot release + engine ordering) on the first matmul of an
    accumulation group. The dedicated Ldweights directly preceding the matmul
    runs on the same in-order PE queue and virtually never carries a wait, so
    shifting the surplus waits onto it preserves ordering semantics."""
    for fn in nc.m.functions:
        for blk in fn.blocks:
            insts = list(blk.instructions)
            for idx, inst in enumerate(insts):
                if type(inst).__name__ != "InstMatmult":
                    continue
                si = inst.sync_info
                waits = list(si.on_wait or []) if si else []
                if len(waits) <= 1:
                    continue
                prev = insts[idx - 1] if idx else None
                assert prev is not None and type(prev).__name__ == "InstLdweights", (
                    f"matmul {inst.name} has {len(waits)} waits but no "
                    f"preceding Ldweights (got {type(prev).__name__})")
                _shift_waits(inst, si, waits, prev)


def _shift_waits(inst, si, waits, carrier):
    psi = carrier.sync_info
    pwaits = list(psi.on_wait or []) if psi else []
    assert len(pwaits) + len(waits) - 1 <= 3, (
        f"{inst.name}: too many combined waits on carrier {carrier.name}")
    moved, kept = waits[:-1], waits[-1:]
    if psi is None:
        carrier.sync_info = type(si)(on_wait=moved, on_update=[])
    else:
        psi.on_wait = pwaits + moved
    si.on_wait = kept


def _rebalance_dma_waits(nc):
    """Same single-wait limit applies to HWDGE DMACopy / gpsimd DMA-direct
    instructions. These always read an SBUF tile written by a producer
    (DVE copy / reciprocal) a few instructions earlier; the producer's
    engine tolerates 3 waits, and since the DMA already waits on the
    producer, conditions moved onto the producer still hold when the DMA
    starts."""
    for fn in nc.m.functions:
        for blk in fn.blocks:
            insts = list(blk.instructions)
            sp_seen = {}   # sem name -> max value already awaited on SP queue
            for idx, inst in enumerate(insts):
                if type(inst).__name__ not in (
                        "InstDMACopy", "InstPartitionBroadcast"):
                    continue
                si = inst.sync_info
                waits = list(si.on_wait or []) if si else []
                is_sp = str(inst.engine) == "EngineType.SP"
                if is_sp and waits:
                    # SP executes serially: waits dominated by an earlier SP
                    # instruction's wait on the same sem are redundant
                    live = [w for w in waits
                            if sp_seen.get(w.ant_name, -1) < w.wait_value]
                    if len(live) < len(waits):
                        si.on_wait = live
                        waits = live
                if is_sp:
                    for w in waits:
                        if sp_seen.get(w.ant_name, -1) < w.wait_value:
                            sp_seen[w.ant_name] = w.wait_value
                if len(waits) <= 1:
                    continue
                src = inst.ins[0].memref if inst.ins else None
                prod = None
                for j in range(idx - 1, max(-1, idx - 400), -1):
                    p = insts[j]
                    pouts = getattr(p, "outs", None)
                    if pouts and pouts[0].memref == src and \
                            type(p).__name__ not in ("InstDMACopy",):
                        prod = p
                        break
                if prod is None:
                    # DRAM load: no producer. SP executes serially, so the
                    # nearest preceding wait-free SP DMA can absorb the
                    # engine-WAR wait; the queue wait stays on this DMA.
                    carrier = None
                    for j in range(idx - 1, max(-1, idx - 400), -1):
                        p = insts[j]
                        if type(p).__name__ == "InstDMACopy" and \
                                str(p.engine) == "EngineType.SP":
                            pw = list(p.sync_info.on_wait or []) \
                                if p.sync_info else []
                            if not pw:
                                carrier = p
                                break
                    if carrier is None:
                        # The engine-WAR wait (kept) implies the slot's
                        # previous DMA write completed (its readers waited on
                        # it), so the same-queue WAW wait is redundant.
                        keep = [w for w in waits if "DMAHW" not in w.ant_name]
                        assert len(keep) == 1, (
                            f"{inst.name}: unexpected pair "
                            f"{[(w.ant_name, w.wait_value) for w in waits]}")
                        si.on_wait = keep
                        continue
                    waits.sort(key=lambda w: 1 if "DMAHW" in w.ant_name else 0)
                    _shift_waits(inst, si, waits, carrier)
                    continue
                # keep the producer-engine wait on the DMA, move the rest
                eng = str(prod.engine)
                key = {"EngineType.DVE": "DVE", "EngineType.ACT": "Activation",
                       "EngineType.Pool": "Pool", "EngineType.PE": "PE",
                       "EngineType.SP": "SP"}.get(eng, "zz")
                waits.sort(key=lambda w: 0 if w.ant_name.startswith(key) else 1)
                waits = waits[::-1]  # producer wait last -> kept
                psi = prod.sync_info
                pn = len(list(psi.on_wait or [])) if psi else 0
                if pn + len(waits) - 1 <= 3:
                    _shift_waits(inst, si, waits, prod)
                else:
                    # producer full: queue wait is FIFO-covered (slot reuse
                    # distance is a multiple of the 8 round-robin queues)
                    keep = [w for w in waits if "DMAHW" not in w.ant_name]
                    assert len(keep) == 1, (
                        f"{inst.name}: unexpected {[(w.ant_name, w.wait_value) for w in waits]}")
                    si.on_wait = keep


def build_program(trace_friendly: bool = False):
    nc = bacc.Bacc("TRN2", target_bir_lowering=False)
    ht = nc.dram_tensor("ht", [D, ST], BF16, kind="ExternalInput")
    wq = nc.dram_tensor("wq", [D, QHD], BF16, kind="ExternalInput")
    wkv = nc.dram_tensor("wkv", [D, 2 * HD], BF16, kind="ExternalInput")
    wo = nc.dram_tensor("wo", [QHD, D], BF16, kind="ExternalInput")
    out = nc.dram_tensor("out", [ST, D], BF16, kind="ExternalOutput")

    with tile.TileContext(nc) as tc:
        with (
            tc.tile_pool(name="singles", bufs=1) as singles,
            tc.tile_pool(name="hstream", bufs=2) as hstream,
            tc.tile_pool(name="expp", bufs=4) as expp,
            tc.tile_pool(name="araw", bufs=2) as arawp,
            tc.tile_pool(name="attn", bufs=2) as attnp,
            tc.tile_pool(name="norm", bufs=2) as normp,
            tc.tile_pool(name="ostage", bufs=4) as ostage,
            tc.tile_pool(name="ps_sc", bufs=2, space="PSUM") as ps_sc,
            tc.tile_pool(name="ps_out", bufs=2, space="PSUM") as ps_out,
            tc.tile_pool(name="ps_op", bufs=2, space="PSUM") as ps_op,
        ):
            # ---- resident weights ----
            # Load order matters for the lead-in: wkv (needed by the first
            # matmul) goes first; the first h chunk is DMA'd right after in
            # proj_phase; wq follows; wo is only needed once the first
            # o_proj group runs (one full job later), so it loads last.
            wq_sb = singles.tile([128, DC, QHD], BF16)
            wkv_sb = singles.tile([128, DC, 2 * HD], BF16)
            wo_sb = singles.tile([128, 2, D], BF16)
            for dc in range(DC):
                nc.sync.dma_start(wkv_sb[:, dc], wkv[dc * 128:(dc + 1) * 128, :])

            # identity for the PE V-transposes
            id_sb = singles.tile([64, HD], BF16)
            masks.make_identity(nc, id_sb[:, :])

            # ---- resident activations (per batch) ----
            # qtdup[h][b]: [128, S], Q^T duplicated on both partition halves
            qtdup = [[singles.tile([128, S], BF16, tag=f"qt{h}_{b}",
                                   name=f"qt{h}_{b}")
                      for b in range(B)] for h in range(QH)]
            # kt2[b]: [128, KP_N, 128]; rows 0:64 = even kb K^T, 64:128 = odd
            kt2 = [singles.tile([128, KP_N, 128], BF16, tag=f"kt{b}",
                                name=f"kt{b}") for b in range(B)]
            vaug = [singles.tile([128, KB_N, HD + 1], BF16, tag=f"vaug{b}",
                                 name=f"vaug{b}") for b in range(B)]
            # V^T staging: rows 64:128 written by DVE (in-partition from
            # PSUM), rows 0:64 filled by SBUF->SBUF DMA; transposed at the
            # end of the projection phase.
            vt_sb = [singles.tile([128, S], BF16, tag=f"vt{b}", name=f"vt{b}")
                     for b in range(B)]
            for b in range(B):
                nc.vector.memset(vaug[b][:, :, HD:HD + 1], 1.0)

            # pending o_proj work from the previous attention job: a list of
            # closures, each one (2 matmuls + a DVE cast [+ DMA]).
            pending = []

            def run_pending(n):
                for _ in range(min(n, len(pending))):
                    pending.pop(0)()

            def proj_phase(b, pieces=None):
                """Emit batch-b projection work. With pieces=None it runs
                inline; otherwise 16 closures are appended to `pieces` so the
                caller can interleave them into attention jobs (one per
                head). h chunks are prefetched one s-chunk ahead; the first
                load fires at the point proj_phase is called."""
                emit = (lambda f: f()) if pieces is None else pieces.append
                h_tiles = {}

                def load(sc, b=b):
                    if sc >= SC_N:
                        return
                    h_sb = hstream.tile([128, DC, 512], BF16, tag="h")
                    h_tiles[sc] = h_sb
                    scol = sc * 512
                    for dc in range(DC):
                        nc.sync.dma_start(
                            h_sb[:, dc],
                            ht[dc * 128:(dc + 1) * 128,
                               b * S + scol:b * S + scol + 512])

                load(0)

                def sc_kv(sc, b=b):
                    scol = sc * 512
                    load(sc + 1)
                    h_sb = h_tiles.pop(sc)
                    sc_kv.h_sb = h_sb
                    pkv = ps_op.tile([128, 512], F32, tag="po")
                    for dc in range(DC):
                        nc.tensor.matmul(pkv, wkv_sb[:, dc], h_sb[:, dc],
                                         start=(dc == 0), stop=(dc == DC - 1))
                    # K^T rows 0:64 -> kt2: kb 4sc+j; even j -> low half
                    # (in-partition), odd j -> high half (cross-partition,
                    # the DVE read-low/write-high direction the v1 kernel
                    # already exercised).
                    for j in range(4):
                        kp, half = (4 * sc + j) // 2, (4 * sc + j) % 2
                        nc.vector.tensor_copy(
                            kt2[b][64 * half:64 * half + 64, kp, :],
                            pkv[0:64, j * 128:(j + 1) * 128])
                    # V^T rows 64:128 -> staging high half, then DMA down
                    nc.vector.tensor_copy(
                        vt_sb[b][64:128, scol:scol + 512], pkv[64:128, :])
                    nc.sync.dma_start(
                        vt_sb[b][0:64, scol:scol + 512],
                        vt_sb[b][64:128, scol:scol + 512])

                def sc_q(sc, m, b=b):
                    scol = sc * 512
                    h_sb = sc_kv.h_sb
                    pq = ps_op.tile([128, 512], F32, tag="po")
                    for dc in range(DC):
                        nc.tensor.matmul(
                            pq, wq_sb[:, dc, m * 128:(m + 1) * 128],
                            h_sb[:, dc],
                            start=(dc == 0), stop=(dc == DC - 1))
                    h0, h1 = 2 * m, 2 * m + 1
                    nc.vector.tensor_copy(
                        qtdup[h0][b][0:64, scol:scol + 512], pq[0:64, :])
                    nc.vector.tensor_copy(
                        qtdup[h1][b][64:128, scol:scol + 512], pq[64:128, :])
                    # duplicate onto the other partition half (DMA)
                    nc.sync.dma_start(
                        qtdup[h0][b][64:128, scol:scol + 512],
                        qtdup[h0][b][0:64, scol:scol + 512])
                    nc.sync.dma_start(
                        qtdup[h1][b][0:64, scol:scol + 512],
                        qtdup[h1][b][64:128, scol:scol + 512])

                def vtrans(g, b=b):
                    tr = ps_op.tile([128, 4, HD], BF16, tag="po")
                    for j in range(4):
                        kb = 4 * g + j
                        nc.tensor.transpose(
                            tr[:, j, :],
                            vt_sb[b][0:64, kb * 128:(kb + 1) * 128],
                            id_sb[:, :])
                    nc.vector.tensor_copy(
                        vaug[b][:, 4 * g:4 * g + 4, 0:HD], tr[:, :, :])

                for sc in range(SC_N):
                    emit(functools.partial(sc_kv, sc))
                    emit(functools.partial(sc_q, sc, 0))
                    emit(functools.partial(sc_q, sc, 1))
                for g in range(4):
                    emit(functools.partial(vtrans, g))

            def attn_job(b, qj, pieces, last=False):
                q0 = qj * 512
                araw = arawp.tile([65, QH, 512], BF16)
                attn_sb = attnp.tile([128, 2, 512], BF16)
                for h in range(QH):
                    outp = ps_out.tile([HD + 1, 512], F32)
                    for kp in range(KP_N):
                        scp = ps_sc.tile([128, 1024], F32)
                        nc.tensor.matmul(
                            scp[:, 0:512], kt2[b][0:64, kp, :],
                            qtdup[h][b][0:64, q0:q0 + 512],
                            start=True, stop=True)
                        nc.tensor.matmul(
                            scp[:, 512:1024], kt2[b][64:128, kp, :],
                            qtdup[h][b][64:128, q0:q0 + 512],
                            start=True, stop=True)
                        expT = expp.tile([128, 1024], BF16)
                        nc.scalar.activation(
                            expT[:, :], scp[:, :],
                            mybir.ActivationFunctionType.Exp, scale=SCALE)
                        nc.tensor.matmul(
                            outp, vaug[b][:, 2 * kp, :], expT[:, 0:512],
                            start=(kp == 0), stop=False)
                        nc.tensor.matmul(
                            outp, vaug[b][:, 2 * kp + 1, :], expT[:, 512:1024],
                            start=False, stop=(kp == KP_N - 1))
                        if (h == 0 and kp >= 6) or (h > 0 and kp >= 3):
                            run_pending(1)
                    # drain this head's accumulator to SBUF (frees PSUM fast)
                    nc.vector.tensor_copy(araw[:, h, :], outp)
                    # one batch-1 projection piece per head (b0 jobs only)
                    if pieces:
                        pieces.pop(0)()

                # job-level normalization (off the PE critical path).
                # 1/den as exp(-ln(den)) on ACT: Ln and Exp share one
                # activation table, and the DVE reciprocal is ~6.3 ns/elem
                # on a single partition (12.9 us per job -- measured).
                lnv = normp.tile([1, QH * 512], F32, tag="lnv")
                nc.scalar.activation(lnv, araw[64:65, :, :],
                                     mybir.ActivationFunctionType.Ln)
                recip = normp.tile([1, QH * 512], F32, tag="recip")
                nc.scalar.activation(recip, lnv,
                                     mybir.ActivationFunctionType.Exp,
                                     scale=-1.0)
                bcast = normp.tile([64, QH * 512], F32, tag="bcast")
                nc.gpsimd.partition_broadcast(bcast, recip)
                for h in range(QH):
                    nc.vector.tensor_mul(
                        attn_sb[(h % 2) * 64:(h % 2) * 64 + 64, h // 2, :],
                        araw[0:64, h, :], bcast[:, h * 512:(h + 1) * 512])

                # queue this job's o_proj as pending closures
                for qc in range(4):
                    ost = ostage.tile([128, 2048], BF16)
                    row = b * S + q0 + qc * 128
                    for nb in range(4):
                        def grp(qc=qc, nb=nb, ost=ost, row=row,
                                attn_sb=attn_sb, act_cast=(last and nb % 2)):
                            po = ps_op.tile([128, 512], F32, tag="po")
                            for hh in range(2):
                                nc.tensor.matmul(
                                    po,
                                    attn_sb[:, hh, qc * 128:(qc + 1) * 128],
                                    wo_sb[:, hh, nb * 512:(nb + 1) * 512],
                                    start=(hh == 0), stop=(hh == 1))
                            if act_cast:
                                nc.scalar.copy(
                                    ost[:, nb * 512:(nb + 1) * 512], po)
                            else:
                                nc.vector.tensor_copy(
                                    ost[:, nb * 512:(nb + 1) * 512], po)
                            if nb == 3:
                                for dd in range(2):
                                    nc.sync.dma_start(
                                        out[row:row + 128,
                                            dd * 1024:(dd + 1) * 1024],
                                        ost[:, dd * 1024:(dd + 1) * 1024])
                        pending.append(grp)
                if last:
                    run_pending(len(pending))

            # ================= schedule =================
            # wq rides behind wkv + the first h chunk; wo (first needed by
            # o_proj of job (0,0), a full job later) loads after proj-b0.
            for dc in range(DC):
                nc.sync.dma_start(wq_sb[:, dc], wq[dc * 128:(dc + 1) * 128, :])
            proj_phase(0)
            for hh in range(2):
                nc.sync.dma_start(wo_sb[:, hh], wo[hh * 128:(hh + 1) * 128, :])
            b1_pieces = []
            proj_phase(1, pieces=b1_pieces)
            for qj in range(QJ_N):
                attn_job(0, qj, b1_pieces)
            assert not b1_pieces
            for qj in range(QJ_N):
                attn_job(1, qj, None, last=(qj == QJ_N - 1))
            run_pending(len(pending))
    nc.compile()
    _rebalance_matmul_waits(nc)
    _rebalance_dma_waits(nc)
    return nc


@functools.lru_cache(maxsize=1)
def _get_program():
    return build_program()


def _in_maps(hidden_states, Wq, Wk, Wv, Wo):
    bf = ml_dtypes.bfloat16
    htT = np.ascontiguousarray(
        hidden_states.reshape(ST, D).T.astype(bf))          # [D, B*S]
    in_maps = []
    for c in range(NCORES):
        wkv = np.concatenate(
            [Wk[:, c * HD:(c + 1) * HD], Wv[:, c * HD:(c + 1) * HD]], axis=1)
        in_maps.append({
            "ht": htT,
            "wq": np.ascontiguousarray(Wq[:, c * QHD:(c + 1) * QHD].astype(bf)),
            "wkv": np.ascontiguousarray(wkv.astype(bf)),
            "wo": np.ascontiguousarray(Wo[c * QHD:(c + 1) * QHD, :].astype(bf)),
        })
    return in_maps


def kernel(hidden_states, Wq, Wk, Wv, Wo):
    hidden_states = np.asarray(hidden_states)
    Wq, Wk, Wv, Wo = (np.asarray(x) for x in (Wq, Wk, Wv, Wo))
    in_maps = _in_maps(hidden_states, Wq, Wk, Wv, Wo)
    nc = _get_program()
    res = run_bass_kernel_spmd(nc, in_maps, core_ids=list(range(NCORES)))
    total = res.results[0]["out"].astype(np.float64)
    for c in range(1, NCORES):
        total += res.results[c]["out"].astype(np.float64)
    return total.reshape(B, S, D).astype(np.float32)


# revision 13
# speedup vs baseline: 1.5709x; 1.0792x over previous
"""GQA attention (B=2, S=2048, D=2048, 32 Q heads / 8 KV heads, HD=64) on 8 trn2 cores.

Sharding: tensor-parallel over heads. Core c gets Q heads [4c, 4c+4), KV head c.
Each core computes a full [B*S, D] partial of the output (its 4 heads through
o_proj); the host sums the 8 partials. No collectives.

v2 design (vs the v1 baseline):
  - K and V projections merged into one matmul stream (stationary [wk|wv]
    [128,128]) -> K^T rows 0-63, V^T rows 64-127 of each PSUM tile. V^T is
    turned into natural V via PE transposes (the v1 h-stationary V projection
    was Ldweights-bound).
  - Scores matmuls are row-tiled pairs: contraction is HD=64, so two key
    blocks' K^T stationaries sit on partition halves (kt2[0:64]=even kb,
    kt2[64:128]=odd kb) and the two matmuls run CONCURRENTLY in the PE array
    (tile_position row groups, auto-derived from base partitions). Q is
    duplicated onto both partition halves via SBUF->SBUF DMA.
  - Normalization is job-level and decoupled: PV accumulators (PSUM) are
    drained per head to SBUF by one DVE copy, then recip -> gpsimd
    partition_broadcast -> DVE muls happen off the critical PE path
    (outp bufs=2 so the next head's PV never waits).
  - o_proj of job J is interleaved into job J+1's kb loop (and into the
    batch-1 projection phase) so the PE never idles while ACT does exp.
  - Output is written bf16 (host accumulates partials in f64).
"""

import functools

import numpy as np
import ml_dtypes

import concourse.bacc as bacc
import concourse.bass as bass
import concourse.mybir as mybir
import concourse.tile as tile
from concourse import masks
from concourse.bass_utils import run_bass_kernel_spmd

B, S, D = 2, 2048, 2048
H, KVH, HD = 32, 8, 64
NCORES = 8
QH = H // NCORES            # 4 q heads per core
ST = B * S                  # 4096 flattened rows
QHD = QH * HD               # 256 (q hd dims per core)
SCALE = 1.0 / np.sqrt(HD)

BF16 = mybir.dt.bfloat16
F32 = mybir.dt.float32

DC = D // 128               # 16 contraction chunks
SC_N = S // 512             # 4 s-chunks per batch for projections
KB_N = S // 128             # 16 key blocks per batch
KP_N = KB_N // 2            # 8 key-block pairs
QJ_N = S // 512             # 4 q-jobs of 512 per batch


def _rebalance_matmul_waits(nc):
    """walrus allows only one sync-wait on a Matmult. Tile occasionally emits
    two (psum-slot release + engine ordering) on the first matmul of an
    accumulation group. The dedicated Ldweights directly preceding the matmul
    runs on the same in-order PE queue and virtually never carries a wait, so
    shifting the surplus waits onto it preserves ordering semantics."""
    for fn in nc.m.functions:
        for blk in fn.blocks:
            insts = list(blk.instructions)
            for idx, inst in enumerate(insts):
                if type(inst).__name__ != "InstMatmult":
                    continue
                si = inst.sync_info
                waits = list(si.on_wait or []) if si else []
                if len(waits) <= 1:
                    continue
                prev = insts[idx - 1] if idx else None
                assert prev is not None and type(prev).__name__ == "InstLdweights", (
                    f"matmul {inst.name} has {len(waits)} waits but no "
                    f"preceding Ldweights (got {type(prev).__name__})")
                _shift_waits(inst, si, waits, prev)


def _shift_waits(inst, si, waits, carrier):
    psi = carrier.sync_info
    pwaits = list(psi.on_wait or []) if psi else []
    assert len(pwaits) + len(waits) - 1 <= 3, (
        f"{inst.name}: too many combined waits on carrier {carrier.name}")
    moved, kept = waits[:-1], waits[-1:]
    if psi is None:
        carrier.sync_info = type(si)(on_wait=moved, on_update=[])
    else:
        psi.on_wait = pwaits + moved
    si.on_wait = kept


def _rebalance_dma_waits(nc):
    """Same single-wait limit applies to HWDGE DMACopy / gpsimd DMA-direct
    instructions. These always read an SBUF tile written by a producer
    (DVE copy / reciprocal) a few instructions earlier; the producer's
    engine tolerates 3 waits, and since the DMA already waits on the
    producer, conditions moved onto the producer still hold when the DMA
    starts."""
    for fn in nc.m.functions:
        for blk in fn.blocks:
            insts = list(blk.instructions)
            sp_seen = {}   # sem name -> max value already awaited on SP queue
            for idx, inst in enumerate(insts):
                if type(inst).__name__ not in (
                        "InstDMACopy", "InstPartitionBroadcast"):
                    continue
                si = inst.sync_info
                waits = list(si.on_wait or []) if si else []
                is_sp = str(inst.engine) == "EngineType.SP"
                if is_sp and waits:
                    # SP executes serially: waits dominated by an earlier SP
                    # instruction's wait on the same sem are redundant
                    live = [w for w in waits
                            if sp_seen.get(w.ant_name, -1) < w.wait_value]
                    if len(live) < len(waits):
                        si.on_wait = live
                        waits = live
                if is_sp:
                    for w in waits:
                        if sp_seen.get(w.ant_name, -1) < w.wait_value:
                            sp_seen[w.ant_name] = w.wait_value
                if len(waits) <= 1:
                    continue
                src = inst.ins[0].memref if inst.ins else None
                prod = None
                for j in range(idx - 1, max(-1, idx - 400), -1):
                    p = insts[j]
                    pouts = getattr(p, "outs", None)
                    if pouts and pouts[0].memref == src and \
                            type(p).__name__ not in ("InstDMACopy",):
                        prod = p
                        break
                if prod is None:
                    # DRAM load: no producer. SP executes serially, so the
                    # nearest preceding wait-free SP DMA can absorb the
                    # engine-WAR wait; the queue wait stays on this DMA.
                    carrier = None
                    for j in range(idx - 1, max(-1, idx - 400), -1):
                        p = insts[j]
                        if type(p).__name__ == "InstDMACopy" and \
                                str(p.engine) == "EngineType.SP":
                            pw = list(p.sync_info.on_wait or []) \
                                if p.sync_info else []
                            if not pw:
                                carrier = p
                                break
                    if carrier is None:
                        # The engine-WAR wait (kept) implies the slot's
                        # previous DMA write completed (its readers waited on
                        # it), so the same-queue WAW wait is redundant.
                        keep = [w for w in waits if "DMAHW" not in w.ant_name]
                        assert len(keep) == 1, (
                            f"{inst.name}: unexpected pair "
                            f"{[(w.ant_name, w.wait_value) for w in waits]}")
                        si.on_wait = keep
                        continue
                    waits.sort(key=lambda w: 1 if "DMAHW" in w.ant_name else 0)
                    _shift_waits(inst, si, waits, carrier)
                    continue
                # keep the producer-engine wait on the DMA, move the rest
                eng = str(prod.engine)
                key = {"EngineType.DVE": "DVE", "EngineType.ACT": "Activation",
                       "EngineType.Pool": "Pool", "EngineType.PE": "PE",
                       "EngineType.SP": "SP"}.get(eng, "zz")
                waits.sort(key=lambda w: 0 if w.ant_name.startswith(key) else 1)
                waits = waits[::-1]  # producer wait last -> kept
                psi = prod.sync_info
                pn = len(list(psi.on_wait or [])) if psi else 0
                if pn + len(waits) - 1 <= 3:
                    _shift_waits(inst, si, waits, prod)
                else:
                    # producer full: queue wait is FIFO-covered (slot reuse
                    # distance is a multiple of the 8 round-robin queues)
                    keep = [w for w in waits if "DMAHW" not in w.ant_name]
                    assert len(keep) == 1, (
                        f"{inst.name}: unexpected {[(w.ant_name, w.wait_value) for w in waits]}")
                    si.on_wait = keep


def _pin_act_tables(nc):
    """The act-table pass picks the first table containing each activation's
    function, so a kernel using Exp and Ln thrashes between table 0
    (exp_and_others) and table 5 (natural_log, which lacks exp) — one
    1.28us ACT table load per job boundary. All functions this kernel uses
    (Exp, Ln, Copy) live together in 'natural_log_exp_and_others', so pin
    the first load to that table and drop the rest (they carry no waits or
    semaphore updates)."""
    from concourse.hw_specs import get_activation_tables

    tables = get_activation_tables(nc.m.arch)
    nl_id = list(tables).index("natural_log_exp_and_others")
    fns = tables["natural_log_exp_and_others"]
    for need in (mybir.ActivationFunctionType.Exp,
                 mybir.ActivationFunctionType.Ln,
                 mybir.ActivationFunctionType.Copy):
        assert need in fns, need
    for fn in nc.m.functions:
        for blk in fn.blocks:
            first = True
            kept = []
            for inst in blk.instructions:
                if isinstance(inst, mybir.InstLoadActFuncSet):
                    si = inst.sync_info
                    assert not (si and (si.on_wait or si.on_update)), inst.name
                    if not first:
                        continue
                    inst.act_func_set_id = nl_id
                    first = False
                kept.append(inst)
            blk.instructions[:] = kept


def build_program(trace_friendly: bool = False):
    nc = bacc.Bacc("TRN2", target_bir_lowering=False)
    ht = nc.dram_tensor("ht", [D, ST], BF16, kind="ExternalInput")
    wq = nc.dram_tensor("wq", [D, QHD], BF16, kind="ExternalInput")
    wkv = nc.dram_tensor("wkv", [D, 2 * HD], BF16, kind="ExternalInput")
    wo = nc.dram_tensor("wo", [QHD, D], BF16, kind="ExternalInput")
    out = nc.dram_tensor("out", [ST, D], BF16, kind="ExternalOutput")

    with tile.TileContext(nc) as tc:
        with (
            tc.tile_pool(name="singles", bufs=1) as singles,
            tc.tile_pool(name="hstream", bufs=2) as hstream,
            tc.tile_pool(name="expp", bufs=4) as expp,
            tc.tile_pool(name="araw", bufs=2) as arawp,
            tc.tile_pool(name="attn", bufs=2) as attnp,
            tc.tile_pool(name="norm", bufs=2) as normp,
            tc.tile_pool(name="ostage", bufs=4) as ostage,
            tc.tile_pool(name="ps_sc", bufs=2, space="PSUM") as ps_sc,
            tc.tile_pool(name="ps_out", bufs=2, space="PSUM") as ps_out,
            tc.tile_pool(name="ps_op", bufs=2, space="PSUM") as ps_op,
        ):
            # ---- resident weights ----
            # Load order matters for the lead-in: wkv (needed by the first
            # matmul) goes first; the first h chunk is DMA'd right after in
            # proj_phase; wq follows; wo is only needed once the first
            # o_proj group runs (one full job later), so it loads last.
            wq_sb = singles.tile([128, DC, QHD], BF16)
            wkv_sb = singles.tile([128, DC, 2 * HD], BF16)
            wo_sb = singles.tile([128, 2, D], BF16)
            nc.sync.dma_start(
                wkv_sb[:, :, :],
                wkv[:, :].rearrange("(a p) j -> p a j", p=128))

            # identity for the PE V-transposes
            id_sb = singles.tile([64, HD], BF16)
            masks.make_identity(nc, id_sb[:, :])

            # ---- resident activations (per batch) ----
            # qtdup[h][b]: [128, S], Q^T duplicated on both partition halves
            qtdup = [[singles.tile([128, S], BF16, tag=f"qt{h}_{b}",
                                   name=f"qt{h}_{b}")
                      for b in range(B)] for h in range(QH)]
            # kt2[b]: [128, KP_N, 128]; rows 0:64 = even kb K^T, 64:128 = odd
            kt2 = [singles.tile([128, KP_N, 128], BF16, tag=f"kt{b}",
                                name=f"kt{b}") for b in range(B)]
            vaug = [singles.tile([128, KB_N, HD + 1], BF16, tag=f"vaug{b}",
                                 name=f"vaug{b}") for b in range(B)]
            # V^T staging: rows 64:128 written by DVE (in-partition from
            # PSUM), rows 0:64 filled by SBUF->SBUF DMA; transposed at the
            # end of the projection phase.
            vt_sb = [singles.tile([128, S], BF16, tag=f"vt{b}", name=f"vt{b}")
                     for b in range(B)]
            for b in range(B):
                nc.vector.memset(vaug[b][:, :, HD:HD + 1], 1.0)

            # pending o_proj work from the previous attention job: a list of
            # closures, each one (2 matmuls + a DVE cast [+ DMA]).
            pending = []

            def run_pending(n):
                for _ in range(min(n, len(pending))):
                    pending.pop(0)()

            def proj_phase(b, pieces=None):
                """Emit batch-b projection work. With pieces=None it runs
                inline; otherwise 16 closures are appended to `pieces` so the
                caller can interleave them into attention jobs (one per
                head). h chunks are prefetched one s-chunk ahead; the first
                load fires at the point proj_phase is called."""
                emit = (lambda f: f()) if pieces is None else pieces.append
                h_tiles = {}

                def load(sc, b=b):
                    if sc >= SC_N:
                        return
                    h_sb = hstream.tile([128, DC, 512], BF16, tag="h")
                    h_tiles[sc] = h_sb
                    scol = b * S + sc * 512
                    # two halves so the first matmuls start at ~1MB loaded
                    for hf in range(2):
                        nc.sync.dma_start(
                            h_sb[:, 8 * hf:8 * hf + 8, :],
                            ht[1024 * hf:1024 * hf + 1024,
                               scol:scol + 512].rearrange(
                                "(a p) j -> p a j", p=128))

                load(0)

                def sc_kv(sc, b=b):
                    scol = sc * 512
                    load(sc + 1)
                    h_sb = h_tiles.pop(sc)
                    sc_kv.h_sb = h_sb
                    pkv = ps_op.tile([128, 512], F32, tag="po")
                    for dc in range(DC):
                        nc.tensor.matmul(pkv, wkv_sb[:, dc], h_sb[:, dc],
                                         start=(dc == 0), stop=(dc == DC - 1))
                    # K^T rows 0:64 -> kt2: kb 4sc+j; even j -> low half
                    # (in-partition), odd j -> high half (cross-partition,
                    # the DVE read-low/write-high direction the v1 kernel
                    # already exercised).
                    for j in range(4):
                        kp, half = (4 * sc + j) // 2, (4 * sc + j) % 2
                        nc.vector.tensor_copy(
                            kt2[b][64 * half:64 * half + 64, kp, :],
                            pkv[0:64, j * 128:(j + 1) * 128])
                    # V^T rows 64:128 -> staging high half, then DMA down
                    nc.vector.tensor_copy(
                        vt_sb[b][64:128, scol:scol + 512], pkv[64:128, :])
                    nc.sync.dma_start(
                        vt_sb[b][0:64, scol:scol + 512],
                        vt_sb[b][64:128, scol:scol + 512])

                def sc_q(sc, m, b=b):
                    scol = sc * 512
                    h_sb = sc_kv.h_sb
                    pq = ps_op.tile([128, 512], F32, tag="po")
                    for dc in range(DC):
                        nc.tensor.matmul(
                            pq, wq_sb[:, dc, m * 128:(m + 1) * 128],
                            h_sb[:, dc],
                            start=(dc == 0), stop=(dc == DC - 1))
                    h0, h1 = 2 * m, 2 * m + 1
                    nc.vector.tensor_copy(
                        qtdup[h0][b][0:64, scol:scol + 512], pq[0:64, :])
                    nc.vector.tensor_copy(
                        qtdup[h1][b][64:128, scol:scol + 512], pq[64:128, :])
                    # duplicate onto the other partition half (DMA)
                    nc.sync.dma_start(
                        qtdup[h0][b][64:128, scol:scol + 512],
                        qtdup[h0][b][0:64, scol:scol + 512])
                    nc.sync.dma_start(
                        qtdup[h1][b][0:64, scol:scol + 512],
                        qtdup[h1][b][64:128, scol:scol + 512])

                def vtrans(g, b=b):
                    tr = ps_op.tile([128, 4, HD], BF16, tag="po")
                    for j in range(4):
                        kb = 4 * g + j
                        nc.tensor.transpose(
                            tr[:, j, :],
                            vt_sb[b][0:64, kb * 128:(kb + 1) * 128],
                            id_sb[:, :])
                    nc.vector.tensor_copy(
                        vaug[b][:, 4 * g:4 * g + 4, 0:HD], tr[:, :, :])

                for sc in range(SC_N):
                    emit(functools.partial(sc_kv, sc))
                    emit(functools.partial(sc_q, sc, 0))
                    emit(functools.partial(sc_q, sc, 1))
                for g in range(4):
                    emit(functools.partial(vtrans, g))

            def attn_job(b, qj, pieces, finish_prev, last=False):
                q0 = qj * 512
                araw = arawp.tile([65, QH, 512], BF16)
                for h in range(QH):
                    outp = ps_out.tile([HD + 1, 512], F32)
                    for kp in range(KP_N):
                        scp = ps_sc.tile([128, 1024], F32)
                        nc.tensor.matmul(
                            scp[:, 0:512], kt2[b][0:64, kp, :],
                            qtdup[h][b][0:64, q0:q0 + 512],
                            start=True, stop=True)
                        nc.tensor.matmul(
                            scp[:, 512:1024], kt2[b][64:128, kp, :],
                            qtdup[h][b][64:128, q0:q0 + 512],
                            start=True, stop=True)
                        expT = expp.tile([128, 1024], BF16)
                        nc.scalar.activation(
                            expT[:, :], scp[:, :],
                            mybir.ActivationFunctionType.Exp, scale=SCALE)
                        nc.tensor.matmul(
                            outp, vaug[b][:, 2 * kp, :], expT[:, 0:512],
                            start=(kp == 0), stop=False)
                        nc.tensor.matmul(
                            outp, vaug[b][:, 2 * kp + 1, :], expT[:, 512:1024],
                            start=False, stop=(kp == KP_N - 1))
                        # previous job's normalization chain is emitted after
                        # this job's first exp so its ACT ops don't
                        # head-of-line-block the exp stream at the boundary
                        if h == 0 and kp == 0 and finish_prev:
                            finish_prev()
                        if (h == 0 and kp >= 6) or (h > 0 and kp >= 3):
                            run_pending(1)
                    # drain this head's accumulator to SBUF (frees PSUM fast)
                    nc.vector.tensor_copy(araw[:, h, :], outp)
                    # one batch-1 projection piece per head (b0 jobs only)
                    if pieces:
                        pieces.pop(0)()

                def finish(araw=araw, b=b, q0=q0, last=last):
                    # job-level normalization: 1/den as exp(-ln(den)) on ACT
                    # (the DVE reciprocal is ~6.3 ns/elem on one partition:
                    # 12.9us per job, measured in v2).
                    lnv = normp.tile([1, QH * 512], F32, tag="lnv")
                    nc.scalar.activation(lnv, araw[64:65, :, :],
                                         mybir.ActivationFunctionType.Ln)
                    recip = normp.tile([1, QH * 512], F32, tag="recip")
                    nc.scalar.activation(recip, lnv,
                                         mybir.ActivationFunctionType.Exp,
                                         scale=-1.0)
                    bcast = normp.tile([64, QH * 512], F32, tag="bcast")
                    nc.gpsimd.partition_broadcast(bcast, recip)
                    attn_sb = attnp.tile([128, 2, 512], BF16)
                    for h in range(QH):
                        nc.vector.tensor_mul(
                            attn_sb[(h % 2) * 64:(h % 2) * 64 + 64,
                                    h // 2, :],
                            araw[0:64, h, :], bcast[:, h * 512:(h + 1) * 512])

                    # queue this job's o_proj as pending closures
                    for qc in range(4):
                        ost = ostage.tile([128, 2048], BF16)
                        row = b * S + q0 + qc * 128
                        for nb in range(4):
                            def grp(qc=qc, nb=nb, ost=ost, row=row,
                                    attn_sb=attn_sb,
                                    act_cast=(last and nb % 2)):
                                po = ps_op.tile([128, 512], F32, tag="po")
                                for hh in range(2):
                                    nc.tensor.matmul(
                                        po,
                                        attn_sb[:, hh,
                                                qc * 128:(qc + 1) * 128],
                                        wo_sb[:, hh,
                                              nb * 512:(nb + 1) * 512],
                                        start=(hh == 0), stop=(hh == 1))
                                if act_cast:
                                    nc.scalar.copy(
                                        ost[:, nb * 512:(nb + 1) * 512], po)
                                else:
                                    nc.vector.tensor_copy(
                                        ost[:, nb * 512:(nb + 1) * 512], po)
                                if nb == 3:
                                    for dd in range(2):
                                        nc.sync.dma_start(
                                            out[row:row + 128,
                                                dd * 1024:(dd + 1) * 1024],
                                            ost[:, dd * 1024:(dd + 1) * 1024])
                            pending.append(grp)
                return finish

            # ================= schedule =================
            # wq rides behind wkv + the first h chunk; wo (first needed by
            # o_proj of job (0,0), a full job later) loads after proj-b0.
            nc.sync.dma_start(
                wq_sb[:, :, :], wq[:, :].rearrange("(a p) j -> p a j", p=128))
            proj_phase(0)
            nc.sync.dma_start(
                wo_sb[:, :, :], wo[:, :].rearrange("(a p) j -> p a j", p=128))
            b1_pieces = []
            proj_phase(1, pieces=b1_pieces)
            fin = None
            for qj in range(QJ_N):
                fin = attn_job(0, qj, b1_pieces, fin)
            assert not b1_pieces
            for qj in range(QJ_N):
                fin = attn_job(1, qj, None, fin, last=(qj == QJ_N - 1))
            fin()
            run_pending(len(pending))
    nc.compile()
    _pin_act_tables(nc)
    _rebalance_matmul_waits(nc)
    _rebalance_dma_waits(nc)
    return nc


@functools.lru_cache(maxsize=1)
def _get_program():
    return build_program()


def _in_maps(hidden_states, Wq, Wk, Wv, Wo):
    bf = ml_dtypes.bfloat16
    htT = np.ascontiguousarray(
        hidden_states.reshape(ST, D).T.astype(bf))          # [D, B*S]
    in_maps = []
    for c in range(NCORES):
        wkv = np.concatenate(
            [Wk[:, c * HD:(c + 1) * HD], Wv[:, c * HD:(c + 1) * HD]], axis=1)
        in_maps.append({
            "ht": htT,
            "wq": np.ascontiguousarray(Wq[:, c * QHD:(c + 1) * QHD].astype(bf)),
            "wkv": np.ascontiguousarray(wkv.astype(bf)),
            "wo": np.ascontiguousarray(Wo[c * QHD:(c + 1) * QHD, :].astype(bf)),
        })
    return in_maps


def kernel(hidden_states, Wq, Wk, Wv, Wo):
    hidden_states = np.asarray(hidden_states)
    Wq, Wk, Wv, Wo = (np.asarray(x) for x in (Wq, Wk, Wv, Wo))
    in_maps = _in_maps(hidden_states, Wq, Wk, Wv, Wo)
    nc = _get_program()
    res = run_bass_kernel_spmd(nc, in_maps, core_ids=list(range(NCORES)))
    total = res.results[0]["out"].astype(np.float64)
    for c in range(1, NCORES):
        total += res.results[c]["out"].astype(np.float64)
    return total.reshape(B, S, D).astype(np.float32)
